# revision 6
# baseline (speedup 1.0000x reference)
"""CMoE hash-routed expert FFN on 8 NeuronCores (expert-parallel).

Host side (the shard/unshard steps): compute hash routing
e = (token_id % 5099) % 64, first-come slot assignment with capacity 512,
scatter tokens into a per-expert [E, D, C] buffer (transposed), and shard
8 experts to each of the 8 cores along with that core's (transposed)
expert weights.  Device side: per expert
    h  = relu(A @ Wk^T)^2        [C, F]
    kv = h @ Wv^T                [C, D]
    r  = sigmoid(A @ Wr^T)       [C, D]
    out = r * kv
computed entirely in transposed form (contraction dim on SBUF partitions).
The h/kv matmuls are bf16 with fp32 PSUM accumulation; the r matmul runs
in fp8e4m3 DoubleRow mode (2x PE throughput) on absmax-scaled copies of A
and Wr, with the dequant scale folded into the sigmoid's input scale.
Output is stored bf16.  Host gathers each token's slot back out of
[E, D, C] and zeroes dropped tokens.
"""

import numpy as np
import ml_dtypes

import concourse.bass as bass
import concourse.mybir as mybir
import concourse.tile as tile
from concourse import bacc
from concourse.bass import ts
from concourse.bass_utils import run_bass_kernel_spmd

HASH_PRIME = 5099
B, T, D, F, E = 8, 4096, 512, 1792, 64
S = B * T
C = 512  # capacity = max(4, ceil(S/E))
N_CORES = 8
E_LOC = E // N_CORES  # experts per core

BF16 = mybir.dt.bfloat16
FP8 = mybir.dt.float8e4
F32 = mybir.dt.float32

_NC = None  # cached compiled Bass program
LAST_RESULT = None  # BassKernelResults of the most recent run (for test.py)


def _build_nc(sig_scale, e_loc=E_LOC, d=D, f=F, c=C):
    """One SPMD program: each core computes e_loc experts' FFN."""
    kd = d // 128   # contraction tiles over D
    kf = f // 128   # contraction tiles over F
    nc = bacc.Bacc("TRN2", target_bir_lowering=False, debug=False,
                   num_devices=N_CORES)

    a_t = nc.dram_tensor("a_t", [e_loc, 128, kd, c], BF16, kind="ExternalInput")
    a8_t = nc.dram_tensor("a8_t", [e_loc, 128, kd, c], FP8, kind="ExternalInput")
    wk_t = nc.dram_tensor("wk_t", [e_loc, 128, kd, f], BF16, kind="ExternalInput")
    wr8_t = nc.dram_tensor("wr8_t", [e_loc, 128, kd, d], FP8, kind="ExternalInput")
    wv_t = nc.dram_tensor("wv_t", [e_loc, 128, kf, d], BF16, kind="ExternalInput")
    out_t = nc.dram_tensor("out_t", [e_loc, 128, kd, c], BF16, kind="ExternalOutput")

    with tile.TileContext(nc) as tc:
        with (
            tc.tile_pool(name="wts", bufs=2) as wts,
            tc.tile_pool(name="r8", bufs=4) as r8p,
            tc.tile_pool(name="sigp", bufs=3) as sigp,
            tc.tile_pool(name="acts", bufs=2) as acts,
            tc.tile_pool(name="ph", bufs=3, space="PSUM") as ph,
            tc.tile_pool(name="pr", bufs=3, space="PSUM") as pr,
            tc.tile_pool(name="pkv", bufs=2, space="PSUM") as pkv,
        ):
            # Four DMA rings: sync HWDGE (all fp8 r operands early, wv
            # stream, last stores), scalar HWDGE (wk halves), vector
            # HWDGE (at stream), gpsimd SWDGE (output stores).  The
            # scheduler hoists r(e+2) matmuls ahead of h(e)'s tail, so
            # every r operand must be resident early or the in-order PE
            # queue stalls on it; the a8/wr8 tiles get their own bufs=4
            # pool so four experts' worth can be in flight with no
            # buffer-reuse dependency blocking the sync ring.
            tiles = {}
            r8tiles = {}
            sigs = {}

            # Warm the PE (HAM downclocks it when idle; ~3.4us of
            # sustained work brings it back) with matmuls on scratch
            # data while the first input DMAs ramp up.
            warm_l = wts.tile([128, 128], BF16, tag="warm_l")
            warm_r = wts.tile([128, c], BF16, tag="warm_r")
            nc.any.memset(warm_l[:], 0.0)
            nc.any.memset(warm_r[:], 0.0)
            for _ in range(24):
                warm_p = pr.tile([128, c], F32, tag="psr")
                nc.tensor.matmul(warm_p[:], lhsT=warm_l[:], rhs=warm_r[:],
                                 start=True, stop=True)

            def load_r8(e, eng=None):
                eng = eng or nc.sync
                a8 = r8p.tile([128, kd, c], FP8, tag="a8")
                wr8 = r8p.tile([128, kd, d], FP8, tag="wr8")
                r8tiles[e] = (a8, wr8)
                eng.dma_start(a8[:], a8_t[e])
                eng.dma_start(wr8[:], wr8_t[e])

            def load_big(e):
                at = wts.tile([128, kd, c], BF16, tag="at")
                wk = wts.tile([128, kd, f], BF16, tag="wk")
                tiles[e] = (at, wk)
                nc.gpsimd.dma_start(at[:], a_t[e])
                # f-halves: h ft-groups 0..kf/2-1 gate on half 0 only
                half = (kf // 2) * 128
                nc.scalar.dma_start(wk[:, :, :half], wk_t[e][:, :, :half])
                nc.scalar.dma_start(wk[:, :, half:], wk_t[e][:, :, half:])

            def emit_r(e):
                a8, wr8 = r8tiles.pop(e)
                sig = sigp.tile([128, kd, c], F32, tag="sig")
                sigs[e] = sig
                for gt in range(kd):
                    psum_r = pr.tile([128, c], F32, tag="psr")
                    for kt in range(0, kd, 2):
                        nc.tensor.matmul(
                            psum_r[:],
                            lhsT=wr8[:, kt:kt + 2, ts(gt, 128)],
                            rhs=a8[:, kt:kt + 2, :],
                            start=(kt == 0),
                            stop=(kt == kd - 2),
                            perf_mode=mybir.MatmulPerfMode.DoubleRow,
                        )
                    nc.scalar.activation(sig[:, gt, :], psum_r[:],
                                         mybir.ActivationFunctionType.Sigmoid,
                                         scale=float(sig_scale))

            def emit_h_kv(e):
                at, wk = tiles.pop(e)
                wv = wts.tile([128, kf, d], BF16, tag="wv")
                nc.sync.dma_start(wv[:], wv_t[e])

                # h^T[f, c] = (relu(Wk^T.T @ A^T))^2, bf16 for matmul 2
                hb = acts.tile([128, kf, c], BF16, tag="hb")
                for ft in range(kf):
                    psum_h = ph.tile([128, c], F32, tag="psh")
                    for kt in range(kd):
                        nc.tensor.matmul(
                            psum_h[:],
                            lhsT=wk[:, kt, ts(ft, 128)],
                            rhs=at[:, kt, :],
                            start=(kt == 0),
                            stop=(kt == kd - 1),
                        )
                    nc.scalar.activation(hb[:, ft, :], psum_h[:],
                                         mybir.ActivationFunctionType.Relu)
                    nc.vector.tensor_mul(hb[:, ft, :], hb[:, ft, :], hb[:, ft, :])

                # kv^T[dd, c] = Wv^T.T @ h^T ; out = sig * kv
                sig = sigs.pop(e)
                ob = acts.tile([128, kd, c], BF16, tag="ob")
                for dt in range(kd):
                    psum_kv = pkv.tile([128, c], F32, tag="pskv")
                    for ft in range(kf):
                        nc.tensor.matmul(
                            psum_kv[:],
                            lhsT=wv[:, ft, ts(dt, 128)],
                            rhs=hb[:, ft, :],
                            start=(ft == 0),
                            stop=(ft == kf - 1),
                        )
                    # store each d-tile as it finishes; the last expert's
                    # stores ride the by-then-idle sync HWDGE ring in
                    # quarter-column chunks so the final store (and the
                    # end-of-kernel DMA drain) trails the last matmul by
                    # as little as possible
                    if e == e_loc - 1:
                        q = c // 4
                        for ci in range(4):
                            cs = slice(ci * q, (ci + 1) * q)
                            nc.vector.tensor_mul(ob[:, dt, cs], psum_kv[:, cs],
                                                 sig[:, dt, cs])
                            nc.sync.dma_start(out_t[e][:, dt, cs], ob[:, dt, cs])
                    else:
                        nc.vector.tensor_mul(ob[:, dt, :], psum_kv[:], sig[:, dt, :])
                        nc.gpsimd.dma_start(out_t[e][:, dt, :], ob[:, dt, :])

            load_r8(0)
            load_r8(1)
            # e=0 big tiles, with a8/wr8(2,3) slotted onto the scalar
            # ring between wk(0)'s halves: the sync ring ramps slowly in
            # the first ~15us and can't deliver six fp8 tiles before the
            # scheduler's hoisted r(2)/r(3) matmuls want them
            at0 = wts.tile([128, kd, c], BF16, tag="at")
            wk0 = wts.tile([128, kd, f], BF16, tag="wk")
            tiles[0] = (at0, wk0)
            nc.gpsimd.dma_start(at0[:], a_t[0])
            half = (kf // 2) * 128
            nc.scalar.dma_start(wk0[:, :, :half], wk_t[0][:, :, :half])
            if e_loc > 2:
                load_r8(2, nc.scalar)
            if e_loc > 3:
                load_r8(3, nc.scalar)
            nc.scalar.dma_start(wk0[:, :, half:], wk_t[0][:, :, half:])
            emit_r(0)
            if e_loc > 1:
                load_big(1)
                emit_r(1)
            for e in range(e_loc):
                emit_h_kv(e)
                if e + 4 < e_loc:
                    load_r8(e + 4)
                if e + 2 < e_loc:
                    load_big(e + 2)
                    emit_r(e + 2)

    nc.compile()
    return nc


def _route(token_ids):
    tid = token_ids.reshape(S).astype(np.int64)
    e_idx = (tid % HASH_PRIME) % E
    order = np.argsort(e_idx, kind="stable")
    sorted_e = e_idx[order]
    starts = np.searchsorted(sorted_e, np.arange(E))
    pos = np.empty(S, np.int64)
    pos[order] = np.arange(S) - starts[sorted_e]
    kept = pos < C
    return e_idx, pos, kept


def kernel(x, token_ids, Wk, Wr, Wv):
    global _NC, LAST_RESULT

    e_idx, pos, kept = _route(token_ids)

    bf16 = ml_dtypes.bfloat16
    fp8 = ml_dtypes.float8_e4m3
    xf = np.ascontiguousarray(x, dtype=np.float32).reshape(S, D)
    disp_t = np.zeros((E, D, C), np.float32)
    disp_t[e_idx[kept], :, pos[kept]] = xf[kept]


    # fp8 copies for the r matmul, absmax-scaled into e4m3's [-240, 240]
    wr_f = np.asarray(Wr, dtype=np.float32)
    s_a = 240.0 / max(float(np.abs(disp_t).max()), 1e-30)
    s_w = 240.0 / max(float(np.abs(wr_f).max()), 1e-30)
    def perm(arr):  # [E, (ko p), x] -> [E, p, ko, x], contiguous
        E_, K_, X_ = arr.shape
        return np.ascontiguousarray(
            arr.reshape(E_, K_ // 128, 128, X_).transpose(0, 2, 1, 3))

    a_t = perm(disp_t.astype(bf16))
    a8_t = perm(np.clip(disp_t * s_a, -240, 240).astype(fp8))
    wr8_t = perm(np.clip(wr_f.transpose(0, 2, 1) * s_w, -240, 240).astype(fp8))

    if _NC is None:
        _NC = _build_nc(1.0 / (s_a * s_w))

    wk_t = perm(np.asarray(Wk, dtype=np.float32).transpose(0, 2, 1).astype(bf16))
    wv_t = perm(np.asarray(Wv, dtype=np.float32).transpose(0, 2, 1).astype(bf16))

    in_maps = [
        {
            "a_t": a_t[i * E_LOC:(i + 1) * E_LOC],
            "a8_t": a8_t[i * E_LOC:(i + 1) * E_LOC],
            "wk_t": wk_t[i * E_LOC:(i + 1) * E_LOC],
            "wr8_t": wr8_t[i * E_LOC:(i + 1) * E_LOC],
            "wv_t": wv_t[i * E_LOC:(i + 1) * E_LOC],
        }
        for i in range(N_CORES)
    ]

    LAST_RESULT = run_bass_kernel_spmd(_NC, in_maps, list(range(N_CORES)))
    out_p = np.concatenate(
        [np.asarray(LAST_RESULT.results[i]["out_t"], dtype=np.float32)
         for i in range(N_CORES)], axis=0)  # [E, p, ko, C]
    out_t = out_p.transpose(0, 2, 1, 3).reshape(E, D, C)

    yf = out_t[e_idx, :, np.minimum(pos, C - 1)]
    yf[~kept] = 0.0
    return np.ascontiguousarray(yf.reshape(B, T, D), dtype=np.float32)


# revision 7
# speedup vs baseline: 1.0239x; 1.0239x over previous
"""CMoE hash-routed expert FFN on 8 NeuronCores (expert-parallel).

Host side (the shard/unshard steps): compute hash routing
e = (token_id % 5099) % 64, first-come slot assignment with capacity 512,
scatter tokens into a per-expert [E, D, C] buffer (transposed), and shard
8 experts to each of the 8 cores along with that core's (transposed)
expert weights.  Device side: per expert
    h  = relu(A @ Wk^T)^2        [C, F]
    kv = h @ Wv^T                [C, D]
    r  = sigmoid(A @ Wr^T)       [C, D]
    out = r * kv
computed entirely in transposed form (contraction dim on SBUF partitions).
The h/kv matmuls are bf16 with fp32 PSUM accumulation; the r matmul runs
in fp8e4m3 DoubleRow mode (2x PE throughput) on absmax-scaled copies of A
and Wr, with the dequant scale folded into the sigmoid's input scale.
Output is stored bf16.  Host gathers each token's slot back out of
[E, D, C] and zeroes dropped tokens.
"""

import numpy as np
import ml_dtypes

import concourse.bass as bass
import concourse.mybir as mybir
import concourse.tile as tile
from concourse import bacc
from concourse.bass import ts
from concourse.bass_utils import run_bass_kernel_spmd

HASH_PRIME = 5099
B, T, D, F, E = 8, 4096, 512, 1792, 64
S = B * T
C = 512  # capacity = max(4, ceil(S/E))
N_CORES = 8
E_LOC = E // N_CORES  # experts per core

BF16 = mybir.dt.bfloat16
FP8 = mybir.dt.float8e4
F32 = mybir.dt.float32

_NC = None  # cached compiled Bass program
LAST_RESULT = None  # BassKernelResults of the most recent run (for test.py)


def _build_nc(sig_scale, e_loc=E_LOC, d=D, f=F, c=C):
    """One SPMD program: each core computes e_loc experts' FFN."""
    kd = d // 128   # contraction tiles over D
    kf = f // 128   # contraction tiles over F
    nc = bacc.Bacc("TRN2", target_bir_lowering=False, debug=False,
                   num_devices=N_CORES)

    a_t = nc.dram_tensor("a_t", [e_loc, 128, kd, c], BF16, kind="ExternalInput")
    r8_t = nc.dram_tensor("r8_t", [e_loc, 128, kd, c + d], FP8, kind="ExternalInput")
    wk_t = nc.dram_tensor("wk_t", [e_loc, 128, kd, f], BF16, kind="ExternalInput")
    wv_t = nc.dram_tensor("wv_t", [e_loc, 128, kf, d], BF16, kind="ExternalInput")
    out_t = nc.dram_tensor("out_t", [e_loc, 128, kd, c], BF16, kind="ExternalOutput")

    with tile.TileContext(nc) as tc:
        with (
            tc.tile_pool(name="wts", bufs=2) as wts,
            tc.tile_pool(name="r8", bufs=4) as r8p,
            tc.tile_pool(name="sigp", bufs=3) as sigp,
            tc.tile_pool(name="acts", bufs=2) as acts,
            tc.tile_pool(name="ph", bufs=3, space="PSUM") as ph,
            tc.tile_pool(name="pr", bufs=3, space="PSUM") as pr,
            tc.tile_pool(name="pkv", bufs=2, space="PSUM") as pkv,
        ):
            # Four DMA rings: sync HWDGE (all fp8 r operands early, wv
            # stream, last stores), scalar HWDGE (wk halves), vector
            # HWDGE (at stream), gpsimd SWDGE (output stores).  The
            # scheduler hoists r(e+2) matmuls ahead of h(e)'s tail, so
            # every r operand must be resident early or the in-order PE
            # queue stalls on it; the a8/wr8 tiles get their own bufs=4
            # pool so four experts' worth can be in flight with no
            # buffer-reuse dependency blocking the sync ring.
            tiles = {}
            r8tiles = {}
            sigs = {}

            # Warm the PE (HAM downclocks it when idle; ~3.4us of
            # sustained work brings it back) with matmuls on scratch
            # data while the first input DMAs ramp up.
            warm_l = wts.tile([128, 128], BF16, tag="warm_l")
            warm_r = wts.tile([128, c], BF16, tag="warm_r")
            nc.any.memset(warm_l[:], 0.0)
            nc.any.memset(warm_r[:], 0.0)
            for _ in range(20):
                warm_p = pr.tile([128, c], F32, tag="psr")
                nc.tensor.matmul(warm_p[:], lhsT=warm_l[:], rhs=warm_r[:],
                                 start=True, stop=True)

            def load_r8(e):
                r8 = r8p.tile([128, kd, c + d], FP8, tag="r8")
                r8tiles[e] = r8
                nc.sync.dma_start(r8[:], r8_t[e])

            def load_big(e):
                at = wts.tile([128, kd, c], BF16, tag="at")
                wk = wts.tile([128, kd, f], BF16, tag="wk")
                tiles[e] = (at, wk)
                nc.gpsimd.dma_start(at[:], a_t[e])
                # f-halves: h ft-groups 0..kf/2-1 gate on half 0 only
                half = (kf // 2) * 128
                nc.scalar.dma_start(wk[:, :, :half], wk_t[e][:, :, :half])
                nc.scalar.dma_start(wk[:, :, half:], wk_t[e][:, :, half:])

            def emit_r(e):
                r8 = r8tiles.pop(e)
                a8, wr8 = r8[:, :, :c], r8[:, :, c:]
                sig = sigp.tile([128, kd, c], F32, tag="sig")
                sigs[e] = sig
                for gt in range(kd):
                    psum_r = pr.tile([128, c], F32, tag="psr")
                    for kt in range(0, kd, 2):
                        nc.tensor.matmul(
                            psum_r[:],
                            lhsT=wr8[:, kt:kt + 2, ts(gt, 128)],
                            rhs=a8[:, kt:kt + 2, :],
                            start=(kt == 0),
                            stop=(kt == kd - 2),
                            perf_mode=mybir.MatmulPerfMode.DoubleRow,
                        )
                    nc.scalar.activation(sig[:, gt, :], psum_r[:],
                                         mybir.ActivationFunctionType.Sigmoid,
                                         scale=float(sig_scale))

            def emit_h_kv(e):
                at, wk = tiles.pop(e)
                wv = wts.tile([128, kf, d], BF16, tag="wv")
                nc.sync.dma_start(wv[:], wv_t[e])

                # h^T[f, c] = (relu(Wk^T.T @ A^T))^2, bf16 for matmul 2
                hb = acts.tile([128, kf, c], BF16, tag="hb")
                for ft in range(kf):
                    psum_h = ph.tile([128, c], F32, tag="psh")
                    for kt in range(kd):
                        nc.tensor.matmul(
                            psum_h[:],
                            lhsT=wk[:, kt, ts(ft, 128)],
                            rhs=at[:, kt, :],
                            start=(kt == 0),
                            stop=(kt == kd - 1),
                        )
                    nc.scalar.activation(hb[:, ft, :], psum_h[:],
                                         mybir.ActivationFunctionType.Relu)
                    nc.vector.tensor_mul(hb[:, ft, :], hb[:, ft, :], hb[:, ft, :])

                # kv^T[dd, c] = Wv^T.T @ h^T ; out = sig * kv
                sig = sigs.pop(e)
                ob = acts.tile([128, kd, c], BF16, tag="ob")
                for dt in range(kd):
                    psum_kv = pkv.tile([128, c], F32, tag="pskv")
                    for ft in range(kf):
                        nc.tensor.matmul(
                            psum_kv[:],
                            lhsT=wv[:, ft, ts(dt, 128)],
                            rhs=hb[:, ft, :],
                            start=(ft == 0),
                            stop=(ft == kf - 1),
                        )
                    nc.vector.tensor_mul(ob[:, dt, :], psum_kv[:], sig[:, dt, :])
                    # store each d-tile as it finishes; the last expert's
                    # stores ride the by-then-idle sync HWDGE ring (lower
                    # latency than SWDGE) to shorten the kernel tail
                    dst = out_t[e][:, dt, :]
                    if e == e_loc - 1:
                        nc.sync.dma_start(dst, ob[:, dt, :])
                    else:
                        nc.gpsimd.dma_start(dst, ob[:, dt, :])

            for e in range(min(4, e_loc)):
                load_r8(e)
            load_big(0)
            emit_r(0)
            if e_loc > 1:
                load_big(1)
                emit_r(1)
            for e in range(e_loc):
                emit_h_kv(e)
                if e + 4 < e_loc:
                    load_r8(e + 4)
                if e + 2 < e_loc:
                    load_big(e + 2)
                    emit_r(e + 2)

    nc.compile()
    return nc


def _route(token_ids):
    tid = token_ids.reshape(S).astype(np.int64)
    e_idx = (tid % HASH_PRIME) % E
    order = np.argsort(e_idx, kind="stable")
    sorted_e = e_idx[order]
    starts = np.searchsorted(sorted_e, np.arange(E))
    pos = np.empty(S, np.int64)
    pos[order] = np.arange(S) - starts[sorted_e]
    kept = pos < C
    return e_idx, pos, kept


def kernel(x, token_ids, Wk, Wr, Wv):
    global _NC, LAST_RESULT

    e_idx, pos, kept = _route(token_ids)

    bf16 = ml_dtypes.bfloat16
    fp8 = ml_dtypes.float8_e4m3
    xf = np.ascontiguousarray(x, dtype=np.float32).reshape(S, D)
    disp_t = np.zeros((E, D, C), np.float32)
    disp_t[e_idx[kept], :, pos[kept]] = xf[kept]


    # fp8 copies for the r matmul, absmax-scaled into e4m3's [-240, 240]
    wr_f = np.asarray(Wr, dtype=np.float32)
    s_a = 240.0 / max(float(np.abs(disp_t).max()), 1e-30)
    s_w = 240.0 / max(float(np.abs(wr_f).max()), 1e-30)
    def perm(arr):  # [E, (ko p), x] -> [E, p, ko, x], contiguous
        E_, K_, X_ = arr.shape
        return np.ascontiguousarray(
            arr.reshape(E_, K_ // 128, 128, X_).transpose(0, 2, 1, 3))

    a_t = perm(disp_t.astype(bf16))
    r8_t = np.concatenate([
        perm(np.clip(disp_t * s_a, -240, 240).astype(fp8)),
        perm(np.clip(wr_f.transpose(0, 2, 1) * s_w, -240, 240).astype(fp8)),
    ], axis=3)

    if _NC is None:
        _NC = _build_nc(1.0 / (s_a * s_w))

    wk_t = perm(np.asarray(Wk, dtype=np.float32).transpose(0, 2, 1).astype(bf16))
    wv_t = perm(np.asarray(Wv, dtype=np.float32).transpose(0, 2, 1).astype(bf16))

    in_maps = [
        {
            "a_t": a_t[i * E_LOC:(i + 1) * E_LOC],
            "r8_t": r8_t[i * E_LOC:(i + 1) * E_LOC],
            "wk_t": wk_t[i * E_LOC:(i + 1) * E_LOC],
            "wv_t": wv_t[i * E_LOC:(i + 1) * E_LOC],
        }
        for i in range(N_CORES)
    ]

    LAST_RESULT = run_bass_kernel_spmd(_NC, in_maps, list(range(N_CORES)))
    out_p = np.concatenate(
        [np.asarray(LAST_RESULT.results[i]["out_t"], dtype=np.float32)
         for i in range(N_CORES)], axis=0)  # [E, p, ko, C]
    out_t = out_p.transpose(0, 2, 1, 3).reshape(E, D, C)

    yf = out_t[e_idx, :, np.minimum(pos, C - 1)]
    yf[~kept] = 0.0
    return np.ascontiguousarray(yf.reshape(B, T, D), dtype=np.float32)
